# revision 1
# baseline (speedup 1.0000x reference)
"""Adaptive thresholding (11x11 box mean, BORDER_REPLICATE, THRESH_BINARY_INV)
on 8 TRN2 NeuronCores, data-parallel over the batch dim (16 images/core).

Per 512x512 image (fp16 data path), 4 row-blocks of 128 rows:
  - x DMA'd as fp16 into ximg [128, 4, 533]: per segment, cols 0..10 zeros,
    11..15 left margin (edge replicate), 16..527 x rows, 528..532 right margin.
    A second input plane xlo (fp8e4m3) carries 121*(x_f32 - fp16(x)) so the
    compare sees x at ~fp24 precision.
  - ONE DVE sliding-window scan over the flattened [128, 2121] view:
    state = (xp[t] + state) - xp[t-11] (op0=add, op1=subtract); the 11-col
    zero head between segments self-drains the window state, so segments
    stay independent. Output (fp16) holds the horizontal 11-tap sums W11,
    with segment pos's W11[c] at col pos*533 + 10 + c.
  - PE per block (N=512 into one PSUM bank), matmuls grouped by weight
    matrix across the 4 banks to maximize weight reuse:
      main band  BM^T @ W11_seg            (K=128, fp16, weights 1.0)
      identity   (-121*I)^T @ x_seg        (K=128, fp16)
      id low     (-I)^T @ xlo_seg          (K=128, fp8)
      halo next  BHN^T @ W11_nextseg       (K=128, fp16, rows 0..4 nonzero)
      halo prev  BHP^T @ W11_prevseg       (K=128, fp16, rows 123..127)
    PSUM = 121*(mean - x).
  - ACT: Sign(PSUM - 242) -> bf16 {-1,0,+1}, DMA'd out as bf16.
Host: out = (sign >= 0) * 255  (inclusive compare matches x <= mean-2).
"""
import sys
sys.path.insert(0, '/opt/trn_rl_repo')
import numpy as np
import concourse.bass as bass
import concourse.tile as tile
from concourse import bacc, mybir
from concourse.bass_utils import run_bass_kernel_spmd
F32 = mybir.dt.float32
F16 = mybir.dt.float16
BF16 = mybir.dt.bfloat16
F8 = mybir.dt.float8e4

N_CORES = 8
BATCH, H, W = 128, 512, 512
IMGS_PER_CORE = BATCH // N_CORES      # 16
ROWS_PER_CORE = IMGS_PER_CORE * H     # 8192
BLK = 128
NBLK = H // BLK                       # 4
K = 11
PAD = K // 2                          # 5
ZH = K                                # zero head width
WT = ZH + PAD + W + PAD               # 533 segment width
XP0 = ZH                              # xp offset within segment (11)
X0 = ZH + PAD                         # x offset within segment (16)
FLAT = NBLK * WT                      # 2132
SCLEN = FLAT - ZH                     # 2121 scan steps
KH = 32                               # halo row-group size


def _band_matrices(dtype=np.float16):
    r = np.arange(BLK)
    bm_mid = (np.abs(r[:, None] - r[None, :]) <= PAD).astype(dtype)
    bm_top = bm_mid.copy()
    for rr in range(PAD):
        bm_top[0, rr] += dtype(PAD - rr)
    bm_bot = bm_mid.copy()
    for rr in range(BLK - PAD, BLK):
        bm_bot[BLK - 1, rr] += dtype(rr - (BLK - PAD - 1))
    # halo prev: weight rows are prev-segment partitions 96..127 (rel 0..31);
    # partition 96+p is image row (seg base - 32 + p); nonzero for p>=27:
    # row k = -32+p affects output r iff |r - k| <= 5 -> r <= p - 27.
    bhp = np.zeros((BLK, BLK), dtype=dtype)
    for p in range(BLK - PAD, BLK):
        bhp[p, 0:p - (BLK - PAD) + 1] = 1.0
    # halo next: partitions 0..31 of next segment = image rows 128+p;
    # affects r iff r >= 123+p, for p in 0..4.
    bhn = np.zeros((BLK, BLK), dtype=dtype)
    for p in range(PAD):
        bhn[p, BLK - PAD + p:BLK] = 1.0
    idn = (-121.0 * np.eye(BLK)).astype(dtype)
    import ml_dtypes
    idn_lo = (-1.0 * np.eye(BLK)).astype(ml_dtypes.float8_e4m3)
    return {"bm_top": bm_top, "bm_mid": bm_mid, "bm_bot": bm_bot,
            "bhp": bhp, "bhn": bhn, "idn": idn, "idn_lo": idn_lo}


def _build():
    nc = bacc.Bacc(None, target_bir_lowering=False, debug=False)
    x_d = nc.declare_dram_parameter("x", [ROWS_PER_CORE, W], F16, isOutput=False)
    xlo_d = nc.declare_dram_parameter("xlo", [ROWS_PER_CORE, W], F8, isOutput=False)
    shapes = {"bm_top": [BLK, BLK], "bm_mid": [BLK, BLK], "bm_bot": [BLK, BLK],
              "bhp": [BLK, BLK], "bhn": [BLK, BLK], "idn": [BLK, BLK]}
    consts = {nm: nc.declare_dram_parameter(nm, sh, F16, isOutput=False)
              for nm, sh in shapes.items()}
    consts["idn_lo"] = nc.declare_dram_parameter("idn_lo", [BLK, BLK], F8,
                                                 isOutput=False)
    out_d = nc.declare_dram_parameter("out", [ROWS_PER_CORE, W], BF16, isOutput=True)
    xv = x_d[:].rearrange("(i p q) c -> i p q c", p=NBLK, q=BLK)   # [16,4,128,512]
    xlv = xlo_d[:].rearrange("(i p q) c -> i p q c", p=NBLK, q=BLK)
    ov = out_d[:].rearrange("(i p q) c -> i p q c", p=NBLK, q=BLK)

    with tile.TileContext(nc) as tc:
        with (
            tc.tile_pool(name="cpool", bufs=1) as cpool,
            tc.tile_pool(name="xin", bufs=5) as x_pool,
            tc.tile_pool(name="scr", bufs=5) as s_pool,
            tc.tile_pool(name="outp", bufs=3) as o_pool,
            tc.tile_pool(name="psum", bufs=8, space=bass.MemorySpace.PSUM) as ps_pool,
        ):
            ct = {}
            for nm, d in consts.items():
                t = cpool.tile(list(d.shape), d.dtype, tag=nm)
                nc.scalar.dma_start(t[:], d[:])
                ct[nm] = t
            bias_t = cpool.tile([BLK, 1], F32, tag="bias")
            nc.vector.memset(bias_t[:], -242.0)

            imgs = {}  # i -> (ximg, scr_img, oimg)

            def front_img(i):
                ximg = x_pool.tile([BLK, NBLK, WT], F16, tag="ximg")
                nc.sync.dma_start(
                    ximg[:, :, X0:X0 + W],
                    xv[i].rearrange("p q c -> q p c"))
                # image 0: fill margins on DVE itself (idle until the first
                # scan) to skip two cross-engine semaphore hops at startup
                eng = nc.vector if i == 0 else nc.gpsimd
                eng.memset(ximg[:, :, 0:ZH], 0.0)
                eng.tensor_copy(
                    ximg[:, :, XP0:X0],
                    ximg[:, :, X0:X0 + 1].to_broadcast((BLK, NBLK, PAD)))
                eng.tensor_copy(
                    ximg[:, :, X0 + W:WT],
                    ximg[:, :, X0 + W - 1:X0 + W].to_broadcast((BLK, NBLK, PAD)))
                xlo = x_pool.tile([BLK, NBLK, W], F8, tag="xlo")
                nc.gpsimd.dma_start(xlo[:], xlv[i].rearrange("p q c -> q p c"))
                flat = ximg[:].rearrange("q p c -> q (p c)")
                s = s_pool.tile([BLK, SCLEN], F16, tag="scr")
                if i in (0, IMGS_PER_CORE - 1):
                    # per-segment scans: image 0 so the first matmuls start
                    # sooner; last image so the final DVE drain (op_dur-266ns)
                    # gating its matmuls is ~1us instead of ~4.3us
                    for pos in range(NBLK):
                        o0 = pos * WT
                        nc.vector.tensor_tensor_scan(
                            s[:, o0:o0 + WT - ZH],
                            flat[:, o0 + ZH:o0 + WT], flat[:, o0:o0 + WT - ZH],
                            0.0, op0=mybir.AluOpType.add,
                            op1=mybir.AluOpType.subtract)
                else:
                    nc.vector.tensor_tensor_scan(
                        s[:], flat[:, ZH:FLAT], flat[:, 0:SCLEN], 0.0,
                        op0=mybir.AluOpType.add, op1=mybir.AluOpType.subtract)
                oimg = o_pool.tile([BLK, NBLK, W], BF16, tag="oimg")
                imgs[i] = (ximg, xlo, s, oimg)

            def back_img(i):
                ximg, xlo, s, oimg = imgs.pop(i)
                # matmuls grouped by weight matrix across the 4 psum banks so
                # walrus ldw-opt amortizes LDWEIGHTS; per-bank group order:
                # bm (start) -> idn -> bhn -> bhp (last touch carries stop).
                pss = [ps_pool.tile([BLK, W], F32, tag="ps", name=f"ps_{i}_{j}")
                       for j in range(NBLK)]

                def segof(pos):
                    return pos * WT + (K - 1)

                for pos in range(NBLK):
                    sfx = "top" if pos == 0 else ("bot" if pos == NBLK - 1 else "mid")
                    nc.tensor.matmul(pss[pos][:], ct["bm_" + sfx][:],
                                     s[:, segof(pos):segof(pos) + W],
                                     start=True, stop=False)
                for pos in range(NBLK):
                    nc.tensor.matmul(pss[pos][:], ct["idn"][:],
                                     ximg[:, pos, X0:X0 + W],
                                     start=False, stop=False)
                for pos in range(NBLK):
                    nc.tensor.matmul(pss[pos][:], ct["idn_lo"][:],
                                     xlo[:, pos, :],
                                     start=False, stop=False)
                for pos in range(NBLK - 1):
                    # bank 0's accumulation ends here (it gets no bhp term)
                    nc.tensor.matmul(pss[pos][:], ct["bhn"][:],
                                     s[:, segof(pos + 1):segof(pos + 1) + W],
                                     start=False, stop=(pos == 0))
                for pos in range(1, NBLK):
                    nc.tensor.matmul(pss[pos][:], ct["bhp"][:],
                                     s[:, segof(pos - 1):segof(pos - 1) + W],
                                     start=False, stop=True)
                # bank 0's last touch is its bhn (pos==NBLK-2 above): fix stops
                for pos in range(NBLK):
                    nc.scalar.activation(
                        oimg[:, pos, :], pss[pos][:],
                        mybir.ActivationFunctionType.Sign,
                        bias=bias_t[:], scale=1.0)
                    if i == IMGS_PER_CORE - 1:
                        nc.scalar.dma_start(ov[i, pos], oimg[:, pos, :])
                if i != IMGS_PER_CORE - 1:
                    nc.scalar.dma_start(ov[i].rearrange("p q c -> q p c"), oimg[:])

            front_img(0)
            front_img(1)
            front_img(2)
            for i in range(IMGS_PER_CORE):
                back_img(i)
                if i + 3 < IMGS_PER_CORE:
                    front_img(i + 3)
    nc.compile()
    return nc


_NC_CACHE = None


def _make_in_maps(x: np.ndarray) -> list:
    import ml_dtypes
    x = np.asarray(x, dtype=np.float32)
    x16 = x.reshape(BATCH, H, W).astype(np.float16)
    xlo16 = (121.0 * (x.reshape(BATCH, H, W) - x16.astype(np.float32))).astype(
        ml_dtypes.float8_e4m3)
    consts = _band_matrices()
    in_maps = []
    for c in range(N_CORES):
        shard = x16[c * IMGS_PER_CORE:(c + 1) * IMGS_PER_CORE].reshape(
            ROWS_PER_CORE, W)
        shard_lo = xlo16[c * IMGS_PER_CORE:(c + 1) * IMGS_PER_CORE].reshape(
            ROWS_PER_CORE, W)
        m = {"x": np.ascontiguousarray(shard),
             "xlo": np.ascontiguousarray(shard_lo)}
        m.update(consts)
        in_maps.append(m)
    return in_maps


def kernel(x: np.ndarray) -> np.ndarray:
    global _NC_CACHE
    if _NC_CACHE is None:
        _NC_CACHE = _build()
    nc = _NC_CACHE
    in_maps = _make_in_maps(x)
    res = run_bass_kernel_spmd(nc, in_maps, core_ids=list(range(N_CORES)))
    out = np.empty((BATCH, H, W), dtype=np.float32)
    for c in range(N_CORES):
        sgn = res.results[c]["out"].astype(np.float32)
        out[c * IMGS_PER_CORE:(c + 1) * IMGS_PER_CORE] = \
            ((sgn >= 0.0) * np.float32(255.0)).reshape(IMGS_PER_CORE, H, W)
    return out.reshape(BATCH, H, W, 1)



# revision 2
# speedup vs baseline: 1.0676x; 1.0676x over previous
"""Adaptive thresholding (11x11 box mean, BORDER_REPLICATE, THRESH_BINARY_INV)
on 8 TRN2 NeuronCores, data-parallel over the batch dim (16 images/core).

v2 design (DVE-scan-bound):
  - Host pre-bakes, per image, a [128, 4, 533] fp16 plane holding y = fp16(x)/4
    with an 11-col zero head and 5-col replicate margins per 533-col segment.
    The /4 scaling halves the fp16 rounding noise of the scan output
    (W11 <= 701 -> ulp 0.5) at zero cost. No xlo correction plane: the
    fp16(x) quantization alone keeps rel-err ~1.3e-2 < 2e-2.
  - ONE DVE sliding-window scan per image over the flat [128, 2132] view
    (the zero heads drain the window state between segments). This is the
    bottleneck engine: ~4.6us/image, everything else hides under it.
  - PE per image: 14 matmuls into one [128, 4, 512] f32 PSUM tile (4 banks):
      band    bm_{top,mid,mid,bot}^T @ W11_seg   (vertical 11-sum + edges)
      ident   (-121*I)^T @ y_seg                 (the -121x/4 term)
      halo    bhn/bhp @ W11_{next,prev seg}      (cross-segment vertical halo)
    grouped by weight matrix for LDWEIGHTS reuse.
  - ONE merged Scalar activation per image: Sign(PSUM - 60.5) over all 4
    banks -> fp8e4m3 {-1,0,+1}, DMA'd out as 1 byte/px.
  - All DMA issues on the sync (SP) engine; gpsimd does nothing.
Host: out = (sign >= 0) * 255  (inclusive compare matches x <= mean-2).
"""
import sys
sys.path.insert(0, '/opt/trn_rl_repo')
import numpy as np
import concourse.bass as bass
import concourse.tile as tile
from concourse import bacc, mybir
from concourse.bass_utils import run_bass_kernel_spmd
F32 = mybir.dt.float32
F16 = mybir.dt.float16
F8 = mybir.dt.float8e4

N_CORES = 8
BATCH, H, W = 128, 512, 512
IMGS_PER_CORE = BATCH // N_CORES      # 16
BLK = 128
NBLK = H // BLK                       # 4
K = 11
PAD = K // 2                          # 5
ZH = K                                # zero head width
WT = ZH + PAD + W + PAD               # 533 segment width
X0 = ZH + PAD                         # x offset within segment (16)
FLAT = NBLK * WT                      # 2132
SCLEN = FLAT - ZH                     # 2121 scan steps
ROWS = IMGS_PER_CORE * BLK            # 2048 partition-rows per core


def _band_matrices(dtype=np.float16):
    r = np.arange(BLK)
    bm_mid = (np.abs(r[:, None] - r[None, :]) <= PAD).astype(dtype)
    bm_top = bm_mid.copy()
    for rr in range(PAD):
        bm_top[0, rr] += dtype(PAD - rr)
    bm_bot = bm_mid.copy()
    for rr in range(BLK - PAD, BLK):
        bm_bot[BLK - 1, rr] += dtype(rr - (BLK - PAD - 1))
    # halo prev: weight rows are prev-segment partitions 96..127; partition p
    # is image row (seg base - 128 + p); nonzero for p >= 123: row k = -128+p
    # affects output r iff |r - k| <= 5 -> r <= p - 123.
    bhp = np.zeros((BLK, BLK), dtype=dtype)
    for p in range(BLK - PAD, BLK):
        bhp[p, 0:p - (BLK - PAD) + 1] = 1.0
    # halo next: partitions 0..4 of next segment = image rows 128+p;
    # affects r iff r >= 123+p.
    bhn = np.zeros((BLK, BLK), dtype=dtype)
    for p in range(PAD):
        bhn[p, BLK - PAD + p:BLK] = 1.0
    idn = (-121.0 * np.eye(BLK)).astype(dtype)
    return {"bm_top": bm_top, "bm_mid": bm_mid, "bm_bot": bm_bot,
            "bhp": bhp, "bhn": bhn, "idn": idn}


def _build():
    nc = bacc.Bacc(None, target_bir_lowering=False, debug=False)
    x_d = nc.declare_dram_parameter("x", [ROWS, FLAT], F16, isOutput=False)
    consts = {nm: nc.declare_dram_parameter(nm, [BLK, BLK], F16, isOutput=False)
              for nm in ("bm_top", "bm_mid", "bm_bot", "bhp", "bhn", "idn")}
    out_d = nc.declare_dram_parameter("out", [ROWS, NBLK * W], F8, isOutput=True)

    with tile.TileContext(nc) as tc:
        with (
            tc.tile_pool(name="cpool", bufs=1) as cpool,
            tc.tile_pool(name="xin", bufs=4) as x_pool,
            tc.tile_pool(name="scr", bufs=4) as s_pool,
            tc.tile_pool(name="outp", bufs=3) as o_pool,
            tc.tile_pool(name="psum", bufs=2, space=bass.MemorySpace.PSUM) as ps_pool,
        ):
            ct = {}
            for nm, d in consts.items():
                t = cpool.tile([BLK, BLK], F16, tag=nm)
                nc.scalar.dma_start(t[:], d[:])
                ct[nm] = t
            bias_t = cpool.tile([BLK, 1], F32, tag="bias")
            nc.vector.memset(bias_t[:], -242.0 / 4.0)

            imgs = {}

            def front_img(i):
                ximg = x_pool.tile([BLK, FLAT], F16, tag="ximg")
                nc.sync.dma_start(ximg[:], x_d[i * BLK:(i + 1) * BLK, :])
                s = s_pool.tile([BLK, SCLEN], F16, tag="scr")
                if i in (0, IMGS_PER_CORE - 1):
                    # per-segment scans: shortens pipeline fill (first image)
                    # and drain (last image)
                    for pos in range(NBLK):
                        o0 = pos * WT
                        nc.vector.tensor_tensor_scan(
                            s[:, o0:o0 + WT - ZH],
                            ximg[:, o0 + ZH:o0 + WT], ximg[:, o0:o0 + WT - ZH],
                            0.0, op0=mybir.AluOpType.add,
                            op1=mybir.AluOpType.subtract)
                else:
                    nc.vector.tensor_tensor_scan(
                        s[:], ximg[:, ZH:FLAT], ximg[:, 0:SCLEN], 0.0,
                        op0=mybir.AluOpType.add, op1=mybir.AluOpType.subtract)
                imgs[i] = (ximg, s)

            def back_img(i):
                ximg, s = imgs.pop(i)
                ps = ps_pool.tile([BLK, NBLK, W], F32, tag="ps", name=f"ps_{i}")

                def segof(pos):
                    return pos * WT + (K - 1)

                names = ["bm_top", "bm_mid", "bm_mid", "bm_bot"]
                for pos in range(NBLK):
                    nc.tensor.matmul(ps[:, pos, :], ct[names[pos]][:],
                                     s[:, segof(pos):segof(pos) + W],
                                     start=True, stop=False)
                for pos in range(NBLK):
                    o0 = pos * WT + X0
                    nc.tensor.matmul(ps[:, pos, :], ct["idn"][:],
                                     ximg[:, o0:o0 + W],
                                     start=False, stop=False)
                for pos in range(NBLK - 1):
                    # bank 0's accumulation ends here (it gets no bhp term)
                    nc.tensor.matmul(ps[:, pos, :], ct["bhn"][:],
                                     s[:, segof(pos + 1):segof(pos + 1) + W],
                                     start=False, stop=(pos == 0))
                for pos in range(1, NBLK):
                    nc.tensor.matmul(ps[:, pos, :], ct["bhp"][:],
                                     s[:, segof(pos - 1):segof(pos - 1) + W],
                                     start=False, stop=True)
                oimg = o_pool.tile([BLK, NBLK, W], F8, tag="oimg")
                orow = out_d[i * BLK:(i + 1) * BLK, :].rearrange(
                    "q (p c) -> q p c", p=NBLK)
                if i == IMGS_PER_CORE - 1:
                    # per-bank act+dma for a shorter drain
                    for pos in range(NBLK):
                        nc.scalar.activation(
                            oimg[:, pos, :], ps[:, pos, :],
                            mybir.ActivationFunctionType.Sign,
                            bias=bias_t[:], scale=1.0)
                        nc.sync.dma_start(orow[:, pos, :], oimg[:, pos, :])
                else:
                    nc.scalar.activation(
                        oimg[:], ps[:], mybir.ActivationFunctionType.Sign,
                        bias=bias_t[:], scale=1.0)
                    nc.sync.dma_start(orow[:], oimg[:])

            front_img(0)
            front_img(1)
            front_img(2)
            for i in range(IMGS_PER_CORE):
                back_img(i)
                if i + 3 < IMGS_PER_CORE:
                    front_img(i + 3)
    nc.compile()
    return nc


_NC_CACHE = None


def _make_in_maps(x: np.ndarray) -> list:
    x = np.asarray(x, dtype=np.float32)
    y = (x.reshape(BATCH, H, W).astype(np.float16) / np.float16(4.0))
    # [img, q, p, c] with q = row-within-block, p = block
    yq = y.reshape(BATCH, NBLK, BLK, W).transpose(0, 2, 1, 3)
    plane = np.zeros((BATCH, BLK, NBLK, WT), dtype=np.float16)
    plane[..., X0:X0 + W] = yq
    plane[..., ZH:X0] = yq[..., 0:1]
    plane[..., X0 + W:WT] = yq[..., W - 1:W]
    consts = _band_matrices()
    in_maps = []
    for c in range(N_CORES):
        shard = plane[c * IMGS_PER_CORE:(c + 1) * IMGS_PER_CORE].reshape(
            ROWS, FLAT)
        m = {"x": np.ascontiguousarray(shard)}
        m.update(consts)
        in_maps.append(m)
    return in_maps


def kernel(x: np.ndarray) -> np.ndarray:
    global _NC_CACHE
    if _NC_CACHE is None:
        _NC_CACHE = _build()
    nc = _NC_CACHE
    in_maps = _make_in_maps(x)
    res = run_bass_kernel_spmd(nc, in_maps, core_ids=list(range(N_CORES)))
    out = np.empty((BATCH, H, W), dtype=np.float32)
    for c in range(N_CORES):
        sgn = np.asarray(res.results[c]["out"]).view(np.uint8)
        o = (sgn < 0x80).astype(np.float32) * np.float32(255.0)
        out[c * IMGS_PER_CORE:(c + 1) * IMGS_PER_CORE] = \
            o.reshape(IMGS_PER_CORE, BLK, NBLK, W).transpose(0, 2, 1, 3).reshape(
                IMGS_PER_CORE, H, W)
    return out.reshape(BATCH, H, W, 1)


# revision 4
# speedup vs baseline: 1.4091x; 1.3199x over previous
"""Adaptive thresholding (11x11 box mean, BORDER_REPLICATE, THRESH_BINARY_INV)
on 8 TRN2 NeuronCores, data-parallel over the batch dim (16 images/core).

v3 design:
  - Host pre-bakes, per image, a [128, 4x533] fp16 plane holding y = fp16(x)/4
    with an 11-col zero head and 5-col replicate margins per segment. The /4
    scaling halves the fp16 rounding noise of the scan output. No xlo plane:
    fp16(x) quantization keeps rel-err ~1.4e-2 < 2e-2.
  - Custom DVE op ADAPT_WSCAN: out = inclusive_scan_add(Src0 - Src1), a
    single-ALU-stage recurrence that runs at 1 elem/cycle (2x the stock
    tensor_tensor_scan, which pays a feedback bubble). One scan per image
    over the flat [128, 2132] view; zero heads drain state between segments.
  - PE per image: 14 matmuls into one [128, 4, 512] f32 PSUM tile, ordered
    idn (x-dependent only, overlaps the scan) -> bm -> bhn -> bhp, grouped
    by weight matrix; walrus ldw-opt (re-enabled via a run_command shim)
    drops redundant LDWEIGHTS.
  - ONE merged Scalar activation per image: Sign(PSUM - 60.5) over 4 banks
    -> fp8e4m3 {-1,0,+1}, DMA'd out at 1 byte/px.
  - x-in and out DMA issues on sync (SP); consts as one merged DMA on scalar.
  - First/last image: chunked DMA + per-segment scans and matmul order to
    shorten pipeline fill/drain.
Host: out = (sign >= 0) * 255  (inclusive compare matches x <= mean-2).
"""
import sys
sys.path.insert(0, '/opt/trn_rl_repo')
import numpy as np
import concourse.bass as bass
import concourse.tile as tile
from concourse import bacc, mybir
import concourse.bass_utils as _bu
from concourse.bass_utils import run_bass_kernel_spmd
from concourse import dve_ops as _dops
from concourse.dve_spec import Spec, Src0, Src1, scan, AluOp, lower
from concourse.dve_spec import _has_src1 as _hs1
from concourse.dve_uop import DveOpSpec

F32 = mybir.dt.float32
F16 = mybir.dt.float16
F8 = mybir.dt.float8e4

N_CORES = 8
BATCH, H, W = 128, 512, 512
IMGS_PER_CORE = BATCH // N_CORES      # 16
BLK = 128
NBLK = H // BLK                       # 4
K = 11
PAD = K // 2                          # 5
ZH = K                                # zero head width
WT = ZH + PAD + W + PAD               # 533 segment width
X0 = ZH + PAD                         # x offset within segment (16)
FLAT = NBLK * WT                      # 2132
SCLEN = FLAT - ZH                     # 2121 scan steps
ROWS = IMGS_PER_CORE * BLK            # 2048 partition-rows per core
CN = ("bm_top", "bm_mid", "bm_bot", "bhp", "bhn", "idn")


def _register_wscan():
    name = "ADAPT_WSCAN"
    if name in _dops._SUB_OPCODE_FOR_NAME:
        return next(o for o in _dops.OPS if o.name == name)
    spec = Spec(
        body=scan(AluOp.ADD, Src0 - Src1),
        reference=lambda in0, in1, s0, s1, imm2: np.cumsum(
            in0.astype(np.float32) - in1.astype(np.float32), axis=-1),
    )
    row = _dops._CUSTOM_DVE_ROW_BASE + len(_dops.OPS)
    _dops._SUB_OPCODE_FOR_NAME[name] = row
    shas = {}
    for ver in ("v3", "v4"):
        tmp = DveOpSpec(name=name, opcode=row, uops=lower(spec, ver=ver),
                        rd1_en=_hs1(spec))
        shas[ver] = tmp.sha(ver)
    op = _dops.DveOp(name, spec, subdim=False, uops_sha=shas)
    _dops.OPS.append(op)
    _dops.CUSTOM_DVE_SPECS[name] = spec
    return op


def _enable_ldw_opt():
    """walrus ldw-opt removes redundant LDWEIGHTS between same-weight
    matmuls; bass_utils hardcodes it off."""
    if getattr(_bu, "_ldw_patched", False):
        return
    orig = _bu.run_command

    def patched(cmd, *a, **k):
        cmd = ["--enable-ldw-opt=true" if c == "--enable-ldw-opt=false" else c
               for c in cmd]
        return orig(cmd, *a, **k)

    _bu.run_command = patched
    _bu._ldw_patched = True


def _band_matrices(dtype=np.float16):
    r = np.arange(BLK)
    bm_mid = (np.abs(r[:, None] - r[None, :]) <= PAD).astype(dtype)
    bm_top = bm_mid.copy()
    for rr in range(PAD):
        bm_top[0, rr] += dtype(PAD - rr)
    bm_bot = bm_mid.copy()
    for rr in range(BLK - PAD, BLK):
        bm_bot[BLK - 1, rr] += dtype(rr - (BLK - PAD - 1))
    bhp = np.zeros((BLK, BLK), dtype=dtype)
    for p in range(BLK - PAD, BLK):
        bhp[p, 0:p - (BLK - PAD) + 1] = 1.0
    bhn = np.zeros((BLK, BLK), dtype=dtype)
    for p in range(PAD):
        bhn[p, BLK - PAD + p:BLK] = 1.0
    idn = (-121.0 * np.eye(BLK)).astype(dtype)
    return {"bm_top": bm_top, "bm_mid": bm_mid, "bm_bot": bm_bot,
            "bhp": bhp, "bhn": bhn, "idn": idn}


def _build():
    wop = _register_wscan()
    nc = bacc.Bacc(None, target_bir_lowering=False, debug=False)
    x_d = nc.declare_dram_parameter("x", [ROWS, FLAT], F16, isOutput=False)
    c_d = nc.declare_dram_parameter("consts", [BLK, len(CN) * BLK], F16,
                                    isOutput=False)
    out_d = nc.declare_dram_parameter("out", [ROWS, NBLK * W], F8, isOutput=True)

    with tile.TileContext(nc) as tc:
        with (
            tc.tile_pool(name="cpool", bufs=1) as cpool,
            tc.tile_pool(name="xin", bufs=4) as x_pool,
            tc.tile_pool(name="scr", bufs=4) as s_pool,
            tc.tile_pool(name="outp", bufs=3) as o_pool,
            tc.tile_pool(name="psum", bufs=2, space=bass.MemorySpace.PSUM) as ps_pool,
        ):
            cbig = cpool.tile([BLK, len(CN) * BLK], F16, tag="consts")
            nc.scalar.dma_start(cbig[:], c_d[:])
            ct = {nm: cbig[:, j * BLK:(j + 1) * BLK] for j, nm in enumerate(CN)}
            bias_t = cpool.tile([BLK, 1], F32, tag="bias")
            nc.vector.memset(bias_t[:], -242.0 / 4.0)

            imgs = {}
            EDGE = (0, IMGS_PER_CORE - 1)

            def front_img(i):
                ximg = x_pool.tile([BLK, NBLK, WT], F16, tag="ximg")
                xrow = x_d[i * BLK:(i + 1) * BLK, :].rearrange(
                    "q (p c) -> q p c", p=NBLK)
                s = s_pool.tile([BLK, SCLEN], F16, tag="scr")
                flat = ximg[:].rearrange("q p c -> q (p c)")
                if i in EDGE:
                    # chunked DMA + per-segment scans: finer-grained deps at
                    # the pipeline's fill (i=0) and drain (i=15) ends
                    for pos in range(NBLK):
                        nc.sync.dma_start(ximg[:, pos, :], xrow[:, pos, :])
                    for pos in range(NBLK):
                        o0 = pos * WT
                        nc.vector._custom_dve(
                            wop, out=s[:, o0:o0 + WT - ZH],
                            in0=flat[:, o0 + ZH:o0 + WT],
                            in1=flat[:, o0:o0 + WT - ZH])
                else:
                    nc.sync.dma_start(ximg[:], xrow[:])
                    nc.vector._custom_dve(
                        wop, out=s[:], in0=flat[:, ZH:FLAT],
                        in1=flat[:, 0:SCLEN])
                imgs[i] = (ximg, s)

            def back_img(i):
                ximg, s = imgs.pop(i)
                ps = ps_pool.tile([BLK, NBLK, W], F32, tag="ps", name=f"ps_{i}")
                flat = ximg[:].rearrange("q p c -> q (p c)")

                def segof(pos):
                    return pos * WT + (K - 1)

                def mm(wname, bank, mv, start, stop):
                    nc.tensor.matmul(ps[:, bank, :], ct[wname], mv,
                                     start=start, stop=stop)

                bmn = ["bm_top", "bm_mid", "bm_mid", "bm_bot"]
                sseg = [s[:, segof(p):segof(p) + W] for p in range(NBLK)]
                xseg = [flat[:, p * WT + X0:p * WT + X0 + W] for p in range(NBLK)]
                # idn depends only on ximg -> overlaps this image's scan
                for pos in range(NBLK):
                    mm("idn", pos, xseg[pos], True, False)
                if i in EDGE:
                    # per-segment availability order; stops: bank b's last op
                    mm("bm_top", 0, sseg[0], False, False)
                    mm("bhp", 1, sseg[0], False, False)
                    mm("bm_mid", 1, sseg[1], False, False)
                    mm("bhn", 0, sseg[1], False, True)
                    mm("bhp", 2, sseg[1], False, False)
                    mm("bm_mid", 2, sseg[2], False, False)
                    mm("bhn", 1, sseg[2], False, True)
                    mm("bhp", 3, sseg[2], False, False)
                    mm("bm_bot", 3, sseg[3], False, True)
                    mm("bhn", 2, sseg[3], False, True)
                else:
                    for pos in range(NBLK):
                        mm(bmn[pos], pos, sseg[pos], False, False)
                    for pos in range(NBLK - 1):
                        mm("bhn", pos, sseg[pos + 1], False, pos == 0)
                    for pos in range(1, NBLK):
                        mm("bhp", pos, sseg[pos - 1], False, True)
                oimg = o_pool.tile([BLK, NBLK, W], F8, tag="oimg")
                orow = out_d[i * BLK:(i + 1) * BLK, :].rearrange(
                    "q (p c) -> q p c", p=NBLK)
                if i == IMGS_PER_CORE - 1:
                    for pos in range(NBLK):
                        nc.scalar.activation(
                            oimg[:, pos, :], ps[:, pos, :],
                            mybir.ActivationFunctionType.Sign,
                            bias=bias_t[:], scale=1.0)
                        nc.sync.dma_start(orow[:, pos, :], oimg[:, pos, :])
                else:
                    nc.scalar.activation(
                        oimg[:], ps[:], mybir.ActivationFunctionType.Sign,
                        bias=bias_t[:], scale=1.0)
                    nc.sync.dma_start(orow[:], oimg[:])

            front_img(0)
            front_img(1)
            front_img(2)
            for i in range(IMGS_PER_CORE):
                back_img(i)
                if i + 3 < IMGS_PER_CORE:
                    front_img(i + 3)
    nc.compile()
    return nc


_NC_CACHE = None


def _make_in_maps(x: np.ndarray) -> list:
    x = np.asarray(x, dtype=np.float32)
    y = (x.reshape(BATCH, H, W).astype(np.float16) / np.float16(4.0))
    yq = y.reshape(BATCH, NBLK, BLK, W).transpose(0, 2, 1, 3)
    plane = np.zeros((BATCH, BLK, NBLK, WT), dtype=np.float16)
    plane[..., X0:X0 + W] = yq
    plane[..., ZH:X0] = yq[..., 0:1]
    plane[..., X0 + W:WT] = yq[..., W - 1:W]
    cm = _band_matrices()
    cbig = np.concatenate([cm[nm] for nm in CN], axis=1)
    in_maps = []
    for c in range(N_CORES):
        shard = plane[c * IMGS_PER_CORE:(c + 1) * IMGS_PER_CORE].reshape(
            ROWS, FLAT)
        in_maps.append({"x": np.ascontiguousarray(shard),
                        "consts": np.ascontiguousarray(cbig)})
    return in_maps


def kernel(x: np.ndarray) -> np.ndarray:
    global _NC_CACHE
    if _NC_CACHE is None:
        _NC_CACHE = _build()
    nc = _NC_CACHE
    in_maps = _make_in_maps(x)
    res = run_bass_kernel_spmd(nc, in_maps, core_ids=list(range(N_CORES)))
    out = np.empty((BATCH, H, W), dtype=np.float32)
    for c in range(N_CORES):
        sgn = np.asarray(res.results[c]["out"]).view(np.uint8)
        o = (sgn < 0x80).astype(np.float32) * np.float32(255.0)
        out[c * IMGS_PER_CORE:(c + 1) * IMGS_PER_CORE] = \
            o.reshape(IMGS_PER_CORE, BLK, NBLK, W).transpose(0, 2, 1, 3).reshape(
                IMGS_PER_CORE, H, W)
    return out.reshape(BATCH, H, W, 1)
